# revision 11
# baseline (speedup 1.0000x reference)
"""Trainium2 Bass kernel for nn_DecoderRNN (highway-LSTM decoder).

Strategy (8 NeuronCores, tensor-parallel over the hidden dimension):
  - H=1024 split into 8 chunks of 128; core c owns hidden units
    [c*128,(c+1)*128) for gate math and recurrent state.
  - Input projection pi runs per-step (N=256) into 3 PSUM banks
    (2 gates packed per bank).  The recurrent projection ps
    ACCUMULATES INTO THE SAME BANKS (start=False), so gate
    pre-activations are read directly from PSUM by the Scalar
    engine (bias folded in) -- no DVE adds, no PSUM->SBUF copies.
  - Optionally the i/f/o sigmoid gates' input projection runs in
    fp8e4 DoubleRow (2 k-chunks per pass).  Weights are scaled by
    64 into e4m3's sweet spot; the matching W_state rows are
    scaled by 64 in bf16 and the Scalar engine applies scale=1/64
    when reading the bank (sigmoid slope <=1/4 keeps the extra
    quantization noise well inside the accuracy budget).
  - h critical path is shortened algebraically:
        h = (hw*mask)*o*tanh(c') + ((1-hw)*mask)*pi5
    with the two mask products precomputed off the critical path.
  - Per-step AllGather of bf16 h chunks runs on the GpSimd queue
    (DMA out + collective + ONE strided gather DMA back), so its
    blocking semaphore never stalls the Sync queue that feeds x
    prefetch.
  - Output projection is K-sharded; partial logits summed on host.
"""
import numpy as np

import concourse.bass as bass
import concourse.bacc as bacc
import concourse.mybir as mybir
import concourse.tile as tile
from concourse.bass_utils import run_bass_kernel_spmd

F32 = mybir.dt.float32
BF16 = mybir.dt.bfloat16
FP8 = mybir.dt.float8e4
AF = mybir.ActivationFunctionType
OP = mybir.AluOpType
DR = mybir.MatmulPerfMode.DoubleRow

T, B, DIN, H, C = 32, 256, 4196, 1024, 151
NCORES = 8
HC = H // NCORES            # 128 hidden units per core
KX = 33                     # ceil(4196/128) input k-chunks
DINP = KX * 128             # 4224 padded input dim
CP = 256                    # padded class dim
KH = 17                     # pi split point for software pipelining
NQ = (KX + 3) // 4          # 9 x-quads per group (8x4 + 1x1)

FP8_GATES = (0, 1, 3)       # i,f,o sigmoid gates via fp8e4 DoubleRow
SW = 512.0                  # fp8 pi weight scale (pre-act scaled 512x)
SWS = 64.0                  # fp8 W_state scale
SH = 8.0                    # fp8 h scale (SWS*SH == SW so banks align)
KP8 = 17                    # fp8 k-pairs (covers 34 chunks, last is zero)
PSP = NCORES // 2           # ps DoubleRow k-pairs

_CACHE = {}


def build_nc(n_steps=T):
    assert n_steps % 2 == 0
    n_grp = n_steps // 2
    fp8_gates = tuple(FP8_GATES)
    bf_gates = tuple(g for g in range(6) if g not in fp8_gates)
    nc = bacc.Bacc("TRN2", target_bir_lowering=False, debug=False,
                   num_devices=NCORES)

    xT = nc.dram_tensor("xT", [n_grp, KX, 128, 2 * B], BF16, kind="ExternalInput")
    w_in = nc.dram_tensor("w_in", [KX, 128, 6 * HC], BF16, kind="ExternalInput")
    w_st = nc.dram_tensor("w_st", [PSP, 128, 2 * 5 * HC], FP8, kind="ExternalInput")
    w_out = nc.dram_tensor("w_out", [128, CP], BF16, kind="ExternalInput")
    bias = nc.dram_tensor("bias", [128, 8], F32, kind="ExternalInput")
    maskT = nc.dram_tensor("maskT", [128, B], F32, kind="ExternalInput")
    c0T = nc.dram_tensor("c0T", [128, B], F32, kind="ExternalInput")
    h0g = nc.dram_tensor("h0g", [128, NCORES * B], FP8, kind="ExternalInput")
    out = nc.dram_tensor("out", [n_steps, 2, 128, CP], F32, kind="ExternalOutput")
    if fp8_gates:
        # pair-chunk layout: tile j holds chunks (2j, 2j+1) side by side
        x8 = nc.dram_tensor("x8", [n_steps, KP8, 128, 2 * B], FP8,
                            kind="ExternalInput")
        w8 = nc.dram_tensor("w8", [KP8, 128, 2 * len(fp8_gates) * HC], FP8,
                            kind="ExternalInput")

    rg = [list(range(NCORES))]

    with tile.TileContext(nc) as tc:
        with (
            tc.tile_pool(name="const", bufs=1) as cpool,
            tc.tile_pool(name="wi", bufs=11) as wip,
            tc.tile_pool(name="w8p", bufs=KP8) as w8p,
            tc.tile_pool(name="xp", bufs=16) as xp,
            tc.tile_pool(name="x8p", bufs=15) as x8p,
            tc.tile_pool(name="gt", bufs=1) as gt,
            tc.tile_pool(name="st", bufs=2) as st,
            tc.tile_pool(name="ht", bufs=2) as htp,
            tc.tile_pool(name="lgs", bufs=4) as lgs,
            tc.tile_pool(name="pi", bufs=2, space="PSUM") as pip,
            tc.tile_pool(name="lg", bufs=2, space="PSUM") as lgp,
            tc.tile_pool(name="dram", bufs=4, space="DRAM") as dram,
        ):
            # ---- resident weights / constants (interleaved with group-0 x
            # so the PE can start within ~2us) ----
            wi_t = []           # 11 tiles of 3 chunks each
            xq = {}             # (group, quad) -> [128, <=2048] tile
            x8q = {}            # (step, pair) -> [128, 512] fp8 tile

            def load_x_group(g):
                for q in range(NQ):
                    kn = min(4, KX - 4 * q)
                    t_ = xp.tile([128, 4 * 2 * B], BF16, tag="x", name=f"x{g}_{q}")
                    nc.sync.dma_start(
                        out=t_[:, :kn * 2 * B],
                        in_=xT[g, 4 * q:4 * q + kn].rearrange("k p c -> p k c"))
                    xq[(g, q)] = t_

            def load_x8_step(t):
                # 4 k-pairs per tile/DMA: few, large Sync issues; ring of 15
                # holds 3 full steps so slots free a step before reuse.
                for q in range((KP8 + 3) // 4):
                    kn = min(4, KP8 - 4 * q)
                    t_ = x8p.tile([128, 4 * 2 * B], FP8, tag="x8",
                                  name=f"x8_{t}_{q}")
                    nc.sync.dma_start(
                        out=t_[:, :kn * 2 * B],
                        in_=x8[t, 4 * q:4 * q + kn].rearrange("k p c -> p k c"))
                    x8q[(t, q)] = t_

            for kk in range(11):
                kn = min(3, KX - 3 * kk)
                w_ = wip.tile([128, 3 * 6 * HC], BF16, tag="wi", name=f"wi{kk}")
                nc.sync.dma_start(
                    out=w_[:, :kn * 6 * HC],
                    in_=w_in[3 * kk:3 * kk + kn].rearrange("k p c -> p k c"))
                wi_t.append(w_)
                if kk < NQ:
                    g0q = xp.tile([128, 4 * 2 * B], BF16, tag="x", name=f"x0_{kk}")
                    kn2 = min(4, KX - 4 * kk)
                    nc.sync.dma_start(
                        out=g0q[:, :kn2 * 2 * B],
                        in_=xT[0, 4 * kk:4 * kk + kn2].rearrange("k p c -> p k c"))
                    xq[(0, kk)] = g0q
            w8_t = []
            if fp8_gates:
                for j in range(KP8):
                    w_ = w8p.tile([128, 2 * len(fp8_gates) * HC], FP8,
                                  tag="w8", name=f"w8_{j}")
                    nc.sync.dma_start(out=w_[:], in_=w8[j])
                    w8_t.append(w_)
                load_x8_step(0)
                load_x8_step(1)

            w_st_sb = cpool.tile([128, PSP * 2 * 5 * HC], FP8)
            for k in range(PSP):
                nc.sync.dma_start(
                    out=w_st_sb[:, k * 1280:(k + 1) * 1280], in_=w_st[k])
            w_out_sb = cpool.tile([128, CP], BF16)
            nc.sync.dma_start(out=w_out_sb[:], in_=w_out[:])
            bias_sb = cpool.tile([128, 8], F32)
            nc.sync.dma_start(out=bias_sb[:], in_=bias[:])
            mask_sb = cpool.tile([128, B], F32)
            nc.sync.dma_start(out=mask_sb[:], in_=maskT[:])
            c_prev = st.tile([128, B], F32, tag="c", name="c_init")
            nc.sync.dma_start(out=c_prev[:], in_=c0T[:])
            ht0 = htp.tile([128, NCORES * B], FP8, tag="ht", name="ht0")
            nc.sync.dma_start(out=ht0[:], in_=h0g[:])
            if n_grp > 1:
                load_x_group(1)

            banks = {}          # step -> [3 psum tiles of [128, 512]]
            ht_of = {0: ht0}
            hr_of = {}

            def wslice(k, g):
                return wi_t[k // 3][:, (k % 3) * 768 + g * 128:
                                    (k % 3) * 768 + (g + 1) * 128]

            def xslice(t, k):
                g, half = t // 2, t % 2
                q, kk = k // 4, k % 4
                base = kk * 2 * B + half * B
                return xq[(g, q)][:, base:base + B]

            def emit_pi(t, klo, khi):
                if t not in banks:
                    banks[t] = [pip.tile([128, 2 * B], F32, tag=f"p{i}",
                                         name=f"pb{t}_{i}") for i in range(3)]
                bk = banks[t]
                # start=True resets has_written for the WHOLE bank, so it
                # must be issued exactly once per bank: on its first MM.
                fresh = {0, 1, 2} if klo == 0 else set()

                def first(bi):
                    if bi in fresh:
                        fresh.discard(bi)
                        return True
                    return False
                # fp8 DoubleRow pairs interleave with bf16 chunks (one DR
                # pair per two bf16 chunks) to balance the in-order
                # LDWEIGHTS queue (DR-heavy) against the MM queue.
                for k in range(klo, khi):
                    if fp8_gates and k % 2 == 0:
                        j = k // 2
                        for gi, g in enumerate(fp8_gates):
                            w_ap = w8_t[j][:, :].rearrange(
                                "p (two c) -> p two c", two=2)[
                                :, :, gi * 128:(gi + 1) * 128]
                            xt8 = x8q[(t, j // 4)]
                            co = (j % 4) * 2 * B
                            x_ap = xt8[:, co:co + 2 * B].rearrange(
                                "p (two b) -> p two b", two=2)
                            nc.tensor.matmul(
                                bk[g // 2][:, (g % 2) * B:(g % 2) * B + B],
                                w_ap, x_ap,
                                start=(j == 0 and first(g // 2)), stop=False,
                                perf_mode=DR, skip_group_check=True)
                    for g in bf_gates:
                        nc.tensor.matmul(
                            bk[g // 2][:, (g % 2) * B:(g % 2) * B + B],
                            wslice(k, g), xslice(t, k),
                            start=(k == 0 and first(g // 2)),
                            stop=(g == 5 and k == KX - 1),
                            skip_group_check=True)

            def emit_ps(t):
                # fp8 DoubleRow: pair j covers h chunks (2j, 2j+1); pair-outer
                # so matmuls start as each pair's gather chunks land; gate
                # order 0,2,4,1,3 avoids back-to-back same-bank PSUM RMW.
                bk = banks[t]
                ht = ht_of[t]
                for j in range(PSP):
                    w_ap = w_st_sb[:, j * 1280:(j + 1) * 1280].rearrange(
                        "p (two c) -> p two c", two=2)
                    x_ap = ht[:, 2 * j * B:(2 * j + 2) * B].rearrange(
                        "p (two b) -> p two b", two=2)
                    for g in (0, 2, 4, 1, 3):
                        nc.tensor.matmul(
                            bk[g // 2][:, (g % 2) * B:(g % 2) * B + B],
                            w_ap[:, :, g * 128:(g + 1) * 128], x_ap,
                            start=False, stop=(j == PSP - 1),
                            perf_mode=DR, skip_group_check=True)

            def bsl(g):
                return bias_sb[:, g:g + 1]

            def emit_step(t):
                """gates straight from PSUM; shortened h critical path."""
                nonlocal c_prev
                bk = banks[t]

                def act(nm, g, fn):
                    p_ = gt.tile([128, B], F32, tag=nm, name=f"{nm}{t}")
                    sc = (1.0 / SW) if g < 5 else 1.0
                    nc.scalar.activation(p_[:], bk[g // 2][:, (g % 2) * B:
                                                           (g % 2) * B + B],
                                         fn, bias=bsl(g), scale=sc)
                    return p_
                i_g = act("i", 0, AF.Sigmoid)
                f_g = act("f", 1, AF.Sigmoid)
                m_i = act("m", 2, AF.Tanh)
                o_g = act("o", 3, AF.Sigmoid)
                hw = act("hw", 4, AF.Sigmoid)
                pi5 = act("p5", 5, AF.Identity)

                t1 = gt.tile([128, B], F32, tag="t1", name=f"t1{t}")
                nc.vector.tensor_mul(t1[:], i_g[:], m_i[:])
                t2 = gt.tile([128, B], F32, tag="t2", name=f"t2{t}")
                nc.vector.tensor_mul(t2[:], f_g[:], c_prev[:])
                c_new = st.tile([128, B], F32, tag="c", name=f"c{t}")
                nc.vector.tensor_add(c_new[:], t1[:], t2[:])
                tm = gt.tile([128, B], F32, tag="tm", name=f"tm{t}")
                nc.scalar.activation(tm[:], c_new[:], AF.Tanh)
                # off-critical-path mask products (fill DVE during tanh)
                hm = gt.tile([128, B], F32, tag="hm", name=f"hm{t}")
                nc.vector.tensor_mul(hm[:], hw[:], mask_sb[:])
                s2 = gt.tile([128, B], F32, tag="s2", name=f"s2{t}")
                nc.vector.tensor_mul(s2[:], o_g[:], hm[:])
                nm_ = gt.tile([128, B], F32, tag="nm", name=f"nm{t}")
                nc.vector.tensor_sub(nm_[:], mask_sb[:], hm[:])
                q2 = gt.tile([128, B], F32, tag="q2", name=f"q2{t}")
                nc.vector.tensor_mul(q2[:], nm_[:], pi5[:])
                t5 = gt.tile([128, B], F32, tag="t5", name=f"t5{t}")
                nc.vector.tensor_mul(t5[:], s2[:], tm[:])
                h_r = st.tile([128, B], BF16, tag="hr", name=f"hr{t}")
                nc.vector.tensor_add(h_r[:], t5[:], q2[:])
                c_prev = c_new
                hr_of[t] = h_r

                if t + 1 < n_steps:
                    h8 = gt.tile([128, B], FP8, tag="h8", name=f"h8{t}")
                    nc.scalar.activation(h8[:], h_r[:], AF.Identity, scale=SH)
                    bin_ = dram.tile([128, B], FP8, tag="bin", name=f"bin{t}")
                    nc.gpsimd.dma_start(out=bin_[:], in_=h8[:])
                    bout = dram.tile([NCORES * 128, B], FP8, tag="bout",
                                     name=f"bout{t}", addr_space="Shared")
                    nc.gpsimd.collective_compute(
                        "AllGather", OP.bypass, replica_groups=rg,
                        ins=[bin_.opt()], outs=[bout.opt()])
                    htn = htp.tile([128, NCORES * B], FP8, tag="ht",
                                   name=f"ht{t + 1}")
                    # per-chunk gather-back on 4 queues: chunk k is a clean
                    # [128,B] copy; subtile deps let ps matmul k start as
                    # soon as its chunk lands instead of after one big
                    # strided 1MB gather (~9us on one queue).
                    qs = (nc.gpsimd, nc.scalar, nc.sync)
                    for k in range(NCORES):
                        qs[k % 3].dma_start(
                            out=htn[:, k * B:(k + 1) * B], in_=bout[k * 128:(k + 1) * 128])
                    ht_of[t + 1] = htn

            def emit_logits(t):
                h_r = hr_of.pop(t)
                lg = lgp.tile([128, 2 * CP], F32, tag="lg", name=f"lg{t}")
                for half in range(2):
                    nc.tensor.matmul(
                        lg[:, half * CP:(half + 1) * CP],
                        h_r[:, half * 128:(half + 1) * 128], w_out_sb[:],
                        start=(half == 0), stop=(half == 1),
                        skip_group_check=True)
                for half in range(2):
                    sb = lgs.tile([128, CP], F32, tag="lgs",
                                  name=f"lgs{t}_{half}")
                    nc.vector.tensor_copy(sb[:], lg[:, half * CP:(half + 1) * CP])
                    nc.sync.dma_start(out=out[t, half], in_=sb[:])

            # ---- software-pipelined main loop ----
            emit_pi(0, 0, KX)
            if n_steps > 1:
                emit_pi(1, 0, KH)
            for t in range(n_steps):
                if fp8_gates and t + 2 < n_steps:
                    load_x8_step(t + 2)          # Sync prefetch (used below)
                if t % 2 == 1 and (t + 3) // 2 < n_grp:
                    load_x_group((t + 3) // 2)   # one full iter of lead
                emit_ps(t)                       # PE: waits gather(t-1)
                if t >= 1:
                    emit_logits(t - 1)           # PE: h(t-1) long ready
                if t + 1 < n_steps:
                    emit_pi(t + 1, KH, KX)       # PE filler / chain cover
                if t + 2 < n_steps:
                    emit_pi(t + 2, 0, KH)
                emit_step(t)                     # ACT/DVE/GpSimd chain
            emit_logits(n_steps - 1)

    nc.compile()
    return nc


def _prep_inputs(x, h0, c0, dropout_mask, W_in, b_in, W_state, b_state,
                 W_out, b_out):
    """Host-side shard + transpose + pad. Returns per-core input maps."""
    import ml_dtypes
    bf16 = ml_dtypes.bfloat16
    f8 = ml_dtypes.float8_e4m3

    def q8(a):
        return np.clip(a, -240, 240).astype(f8)
    fp8_gates = tuple(FP8_GATES)
    n_steps = x.shape[0]
    n_grp = n_steps // 2
    # x [T,B,DIN] -> [T, DINP, B] -> groups [T/2, KX, 128, 2B]
    xp = np.zeros((n_steps, DINP, B), dtype=np.float32)
    xp[:, :DIN, :] = x.transpose(0, 2, 1)
    xg = xp.reshape(n_grp, 2, KX, 128, B).transpose(0, 2, 3, 1, 4)
    xT = np.ascontiguousarray(xg.reshape(n_grp, KX, 128, 2 * B)).astype(bf16)
    if fp8_gates:
        # per-step pair-chunk layout [T, KP8, 128, 2B], chunks (2j, 2j+1)
        x8p = np.zeros((n_steps, KP8 * 256, B), dtype=np.float32)
        x8p[:, :DIN, :] = np.clip(x.transpose(0, 2, 1), -240, 240)
        x8v = x8p.reshape(n_steps, KP8, 2, 128, B).transpose(0, 1, 3, 2, 4)
        x8T = np.ascontiguousarray(
            x8v.reshape(n_steps, KP8, 128, 2 * B)).astype(f8)

    h0g = q8(SH * np.ascontiguousarray(
        h0.T.reshape(NCORES, 128, B).transpose(1, 0, 2).reshape(128, NCORES * B)
    ))

    in_maps = []
    for c in range(NCORES):
        sl = slice(c * HC, (c + 1) * HC)
        # bf16 pi gates that share a PSUM bank with ps (g=2,4) carry the
        # 512x bank scale in bf16; the pure-pi gate 5 stays at 1x.
        wi = np.concatenate([W_in[g * H + c * HC:g * H + (c + 1) * HC]
                             * (SW if g < 5 else 1.0)
                             for g in range(6)], axis=0)  # [768, DIN]
        wip = np.zeros((768, DINP), dtype=np.float32)
        wip[:, :DIN] = wi
        w_in_c = np.ascontiguousarray(wip.T.reshape(KX, 128, 768)).astype(bf16)

        ws = np.concatenate([W_state[g * H + c * HC:g * H + (c + 1) * HC]
                             for g in range(5)], axis=0)  # [640, H]
        # fp8 DoubleRow pair layout: [PSP, 128, (2, 640)]
        w_st_c = q8(np.ascontiguousarray(
            (SWS * ws.T).reshape(PSP, 2, 128, 640).transpose(0, 2, 1, 3)
            .reshape(PSP, 128, 1280)))

        wo = np.zeros((128, CP), dtype=np.float32)
        wo[:, :C] = W_out[:, sl].T
        bias_c = np.zeros((128, 8), dtype=np.float32)
        for g in range(6):
            bias_c[:, g] = b_in[g * H + c * HC:g * H + (c + 1) * HC]
            if g < 5:
                bias_c[:, g] += b_state[g * H + c * HC:g * H + (c + 1) * HC]

        m = {"xT": xT, "w_in": w_in_c,
             "w_st": w_st_c, "w_out": wo.astype(bf16), "bias": bias_c,
             "maskT": np.ascontiguousarray(dropout_mask.T[sl]).astype(np.float32),
             "c0T": np.ascontiguousarray(c0.T[sl]).astype(np.float32),
             "h0g": h0g}
        if fp8_gates:
            # [KP8, 128, 2*len*HC]: pair-major, then gate columns per chunk
            wg = np.concatenate(
                [SW * W_in[g * H + c * HC:g * H + (c + 1) * HC]
                 for g in fp8_gates], axis=0)  # [len*128, DIN]
            wgp = np.zeros((len(fp8_gates) * 128, KP8 * 256), dtype=np.float32)
            wgp[:, :DIN] = wg
            # -> [KP8, 2, 128, len*128] -> [KP8, 128, 2, len*128]
            w8v = wgp.T.reshape(KP8, 2, 128, len(fp8_gates) * 128)
            w8c = np.ascontiguousarray(w8v.transpose(0, 2, 1, 3).reshape(
                KP8, 128, 2 * len(fp8_gates) * 128)).astype(f8)
            m["x8"] = x8T
            m["w8"] = w8c
        in_maps.append(m)
    return in_maps


def kernel(x, h0, c0, dropout_mask, W_in, b_in, W_state, b_state,
           W_out, b_out, _trace=False):
    n_steps = x.shape[0]
    if n_steps not in _CACHE:
        _CACHE[n_steps] = build_nc(n_steps)
    nc = _CACHE[n_steps]
    in_maps = _prep_inputs(np.asarray(x, dtype=np.float32),
                           np.asarray(h0, dtype=np.float32),
                           np.asarray(c0, dtype=np.float32),
                           np.asarray(dropout_mask, dtype=np.float32),
                           np.asarray(W_in, dtype=np.float32),
                           np.asarray(b_in, dtype=np.float32),
                           np.asarray(W_state, dtype=np.float32),
                           np.asarray(b_state, dtype=np.float32),
                           np.asarray(W_out, dtype=np.float32),
                           np.asarray(b_out, dtype=np.float32))
    res = run_bass_kernel_spmd(nc, in_maps, list(range(NCORES)), trace=_trace)
    acc = np.zeros((n_steps, 2, 128, CP), dtype=np.float64)
    for r in res.results:
        acc += r["out"]
    logits = acc.reshape(n_steps, B, CP)[:, :, :C].astype(np.float32)
    logits += np.asarray(b_out, dtype=np.float32)
    kernel.last_result = res
    return logits

